# revision 2
# baseline (speedup 1.0000x reference)
"""BayesianLinear (reparameterized sampling + linear) on 8 TRN2 NeuronCores.

Math:  w = weight_mu + weight_eps * exp(0.5*weight_log_var)   [OUT_F, IN_F]
       b = bias_mu + bias_eps * exp(0.5*bias_log_var)         [OUT_F]
       out = x @ w.T + b                                      [BATCH, OUT_F]

Sharding: 2-way over BATCH x 4-way over OUT_F (8 cores).  Each core gets
K-major (transposed) bf16 copies of its x / weight shards, samples the
weight on device into an SBUF-resident bf16 [IN_F, O_CORE] tensor, and
streams x through the TensorEngine in bf16 (fp32 PSUM accumulate).
Bias is sampled on device in fp32 and fused into the PSUM->SBUF copy.
"""

import numpy as np
import ml_dtypes

BATCH = 8192
IN_F = 4096
OUT_F = 4096
B_SHARDS = 2
O_SHARDS = 4
N_CORES = B_SHARDS * O_SHARDS

B_CORE = BATCH // B_SHARDS   # 4096
O_CORE = OUT_F // O_SHARDS   # 1024

BF16 = ml_dtypes.bfloat16

_PROGRAM_CACHE = {}


def build_program(B_core=B_CORE, O_core=O_CORE, K=IN_F):
    """Build + compile the per-core Bass/Tile program (same NEFF on all cores).

    DRAM parameters (per core):
      xt   [K, B_core]  bf16   x shard, transposed (K-major)
      wmu  [K, O_core]  bf16   weight_mu shard, transposed
      wlv  [K, O_core]  bf16   weight_log_var shard, transposed
      weps [K, O_core]  bf16   weight_eps shard, transposed
      bmu/blv/beps [1, O_core] f32
      out  [B_core, O_core] f32
    """
    import concourse.mybir as mybir
    import concourse.tile as tile
    from concourse import bacc

    assert K % 128 == 0 and B_core % 512 == 0 and O_core % 512 == 0
    KT = K // 128          # contraction tiles
    MT = B_core // 512     # x blocks (512 batch rows each)
    NO = O_core // 512     # output free-dim chunks

    f32 = mybir.dt.float32
    bf16 = mybir.dt.bfloat16
    Exp = mybir.ActivationFunctionType.Exp
    mult = mybir.AluOpType.mult
    add = mybir.AluOpType.add

    nc = bacc.Bacc("TRN2", target_bir_lowering=False, debug=False)

    xt = nc.declare_dram_parameter("xt", [K, B_core], bf16, isOutput=False)
    wmu = nc.declare_dram_parameter("wmu", [K, O_core], bf16, isOutput=False)
    wlv = nc.declare_dram_parameter("wlv", [K, O_core], bf16, isOutput=False)
    weps = nc.declare_dram_parameter("weps", [K, O_core], bf16, isOutput=False)
    bmu = nc.declare_dram_parameter("bmu", [1, O_core], f32, isOutput=False)
    blv = nc.declare_dram_parameter("blv", [1, O_core], f32, isOutput=False)
    beps = nc.declare_dram_parameter("beps", [1, O_core], f32, isOutput=False)
    out = nc.declare_dram_parameter("out", [B_core, O_core], f32, isOutput=True)

    xt_r = xt.ap().rearrange("(kt p) b -> p kt b", p=128)
    out_r = out.ap().rearrange("(mt p) o -> p mt o", p=128)

    with tile.TileContext(nc) as tc:
        with (
            tc.tile_pool(name="wres", bufs=KT) as wres_pool,
            tc.tile_pool(name="wstage", bufs=2) as wstage_pool,
            tc.tile_pool(name="xblk", bufs=3) as xpool,
            tc.tile_pool(name="osb", bufs=4) as opool,
            tc.tile_pool(name="bias", bufs=1) as bias_pool,
            tc.tile_pool(name="psum", bufs=8, space="PSUM") as ppool,
        ):
            # ---- bias: b = bmu + beps * exp(0.5*blv), replicated to 128 parts
            b_mu_t = bias_pool.tile([1, O_core], f32, tag="bmu")
            b_lv_t = bias_pool.tile([1, O_core], f32, tag="blv")
            b_eps_t = bias_pool.tile([1, O_core], f32, tag="beps")
            nc.sync.dma_start(out=b_mu_t[:], in_=bmu.ap())
            nc.sync.dma_start(out=b_lv_t[:], in_=blv.ap())
            nc.sync.dma_start(out=b_eps_t[:], in_=beps.ap())
            b_sd = bias_pool.tile([1, O_core], f32, tag="bsd")
            nc.scalar.activation(b_sd[:], b_lv_t[:], Exp, scale=0.5)
            nc.vector.tensor_tensor(out=b_sd[:], in0=b_sd[:], in1=b_eps_t[:], op=mult)
            nc.vector.tensor_tensor(out=b_sd[:], in0=b_sd[:], in1=b_mu_t[:], op=add)
            bias_rep = bias_pool.tile([128, O_core], f32, tag="brep")
            nc.gpsimd.partition_broadcast(bias_rep[:], b_sd[:])

            # ---- weight sampling: wres[k] = bf16(mu + eps * exp(0.5*lv))
            wres = []
            for k in range(KT):
                smu = wstage_pool.tile([128, O_core], bf16, tag="smu")
                slv = wstage_pool.tile([128, O_core], bf16, tag="slv")
                seps = wstage_pool.tile([128, O_core], bf16, tag="seps")
                nc.sync.dma_start(out=smu[:], in_=wmu.ap()[k * 128:(k + 1) * 128, :])
                nc.sync.dma_start(out=slv[:], in_=wlv.ap()[k * 128:(k + 1) * 128, :])
                nc.sync.dma_start(out=seps[:], in_=weps.ap()[k * 128:(k + 1) * 128, :])
                w_k = wres_pool.tile([128, O_core], bf16, tag="wres")
                nc.scalar.activation(w_k[:], slv[:], Exp, scale=0.5)
                nc.vector.tensor_tensor(out=w_k[:], in0=w_k[:], in1=seps[:], op=mult)
                nc.vector.tensor_tensor(out=w_k[:], in0=w_k[:], in1=smu[:], op=add)
                wres.append(w_k)

            # ---- matmul: out[b, o] = x @ w.T + bias
            for m in range(MT):
                xtile = xpool.tile([128, KT, 512], bf16, tag="xblk")
                nc.sync.dma_start(out=xtile[:], in_=xt_r[:, :, m * 512:(m + 1) * 512])
                psums = [
                    [ppool.tile([128, 512], f32, tag="ps", name=f"ps_{m}_{ms}_{n}")
                     for n in range(NO)]
                    for ms in range(4)
                ]
                for k in range(KT):
                    for ms in range(4):
                        lhsT = xtile[:, k, ms * 128:(ms + 1) * 128]
                        for n in range(NO):
                            nc.tensor.matmul(
                                psums[ms][n][:],
                                lhsT,
                                wres[k][:, n * 512:(n + 1) * 512],
                                start=(k == 0),
                                stop=(k == KT - 1),
                            )
                for ms in range(4):
                    for n in range(NO):
                        osb = opool.tile([128, 512], f32, tag="osb")
                        nc.vector.tensor_tensor(
                            out=osb[:],
                            in0=psums[ms][n][:],
                            in1=bias_rep[:, n * 512:(n + 1) * 512],
                            op=add,
                        )
                        nc.sync.dma_start(
                            out=out_r[:, m * 4 + ms, n * 512:(n + 1) * 512],
                            in_=osb[:],
                        )

    nc.compile()
    return nc


def _get_program():
    key = (B_CORE, O_CORE, IN_F)
    if key not in _PROGRAM_CACHE:
        _PROGRAM_CACHE[key] = build_program(*key)
    return _PROGRAM_CACHE[key]


def make_in_maps(x, weight_mu, weight_log_var, bias_mu, bias_log_var,
                 weight_eps, bias_eps):
    """Shard + transpose + cast the full inputs into 8 per-core input maps."""
    x = np.asarray(x, dtype=np.float32)
    weight_mu = np.asarray(weight_mu, dtype=np.float32)
    weight_log_var = np.asarray(weight_log_var, dtype=np.float32)
    weight_eps = np.asarray(weight_eps, dtype=np.float32)
    bias_mu = np.asarray(bias_mu, dtype=np.float32).reshape(-1)
    bias_log_var = np.asarray(bias_log_var, dtype=np.float32).reshape(-1)
    bias_eps = np.asarray(bias_eps, dtype=np.float32).reshape(-1)

    xt = np.ascontiguousarray(x.astype(BF16).T)              # [IN_F, BATCH]
    wmu_t = np.ascontiguousarray(weight_mu.astype(BF16).T)   # [IN_F, OUT_F]
    wlv_t = np.ascontiguousarray(weight_log_var.astype(BF16).T)
    weps_t = np.ascontiguousarray(weight_eps.astype(BF16).T)

    in_maps = []
    for c in range(N_CORES):
        bi, oi = divmod(c, O_SHARDS)
        bs = slice(bi * B_CORE, (bi + 1) * B_CORE)
        os_ = slice(oi * O_CORE, (oi + 1) * O_CORE)
        in_maps.append({
            "xt": np.ascontiguousarray(xt[:, bs]),
            "wmu": np.ascontiguousarray(wmu_t[:, os_]),
            "wlv": np.ascontiguousarray(wlv_t[:, os_]),
            "weps": np.ascontiguousarray(weps_t[:, os_]),
            "bmu": np.ascontiguousarray(bias_mu[os_]).reshape(1, O_CORE),
            "blv": np.ascontiguousarray(bias_log_var[os_]).reshape(1, O_CORE),
            "beps": np.ascontiguousarray(bias_eps[os_]).reshape(1, O_CORE),
        })
    return in_maps


def gather_output(results):
    out = np.empty((BATCH, OUT_F), dtype=np.float32)
    for c in range(N_CORES):
        bi, oi = divmod(c, O_SHARDS)
        out[bi * B_CORE:(bi + 1) * B_CORE, oi * O_CORE:(oi + 1) * O_CORE] = \
            results[c]["out"]
    return out


def run_on_hw(in_maps, trace=False):
    from concourse.bass_utils import run_bass_kernel_spmd
    nc = _get_program()
    return run_bass_kernel_spmd(nc, in_maps, list(range(N_CORES)), trace=trace)


def kernel(x, weight_mu, weight_log_var, bias_mu, bias_log_var,
           weight_eps, bias_eps):
    in_maps = make_in_maps(x, weight_mu, weight_log_var, bias_mu,
                           bias_log_var, weight_eps, bias_eps)
    res = run_on_hw(in_maps, trace=False)
    return gather_output(res.results)


# revision 3
# speedup vs baseline: 1.0265x; 1.0265x over previous
"""BayesianLinear (reparameterized sampling + linear) on 8 TRN2 NeuronCores.

Math:  w = weight_mu + weight_eps * exp(0.5*weight_log_var)   [OUT_F, IN_F]
       b = bias_mu + bias_eps * exp(0.5*bias_log_var)         [OUT_F]
       out = x @ w.T + b                                      [BATCH, OUT_F]

Sharding: 2-way over BATCH x 4-way over OUT_F (8 cores).  Each core gets
K-major (transposed) bf16 copies of its x / weight shards, samples the
weight on device into an SBUF-resident bf16 [IN_F, O_CORE] tensor, and
streams x through the TensorEngine in bf16 (fp32 PSUM accumulate).
Bias is sampled on device in fp32 and fused into the PSUM->SBUF copy.
"""

import numpy as np
import ml_dtypes

BATCH = 8192
IN_F = 4096
OUT_F = 4096
B_SHARDS = 2
O_SHARDS = 4
N_CORES = B_SHARDS * O_SHARDS

B_CORE = BATCH // B_SHARDS   # 4096
O_CORE = OUT_F // O_SHARDS   # 1024

BF16 = ml_dtypes.bfloat16

_PROGRAM_CACHE = {}


def build_program(B_core=B_CORE, O_core=O_CORE, K=IN_F):
    """Build + compile the per-core Bass/Tile program (same NEFF on all cores).

    DRAM parameters (per core):
      xt   [K, B_core]  bf16   x shard, transposed (K-major)
      wmu  [K, O_core]  bf16   weight_mu shard, transposed
      wlv  [K, O_core]  bf16   weight_log_var shard, transposed
      weps [K, O_core]  bf16   weight_eps shard, transposed
      bmu/blv/beps [1, O_core] f32
      out  [B_core, O_core] f32
    """
    import concourse.mybir as mybir
    import concourse.tile as tile
    from concourse import bacc

    assert K % 128 == 0 and B_core % 512 == 0 and O_core % 512 == 0
    KT = K // 128          # contraction tiles
    MT = B_core // 512     # x blocks (512 batch rows each)
    NO = O_core // 512     # output free-dim chunks

    f32 = mybir.dt.float32
    bf16 = mybir.dt.bfloat16
    Exp = mybir.ActivationFunctionType.Exp
    mult = mybir.AluOpType.mult
    add = mybir.AluOpType.add

    nc = bacc.Bacc("TRN2", target_bir_lowering=False, debug=False)

    xt = nc.declare_dram_parameter("xt", [K, B_core], bf16, isOutput=False)
    wmu = nc.declare_dram_parameter("wmu", [K, O_core], bf16, isOutput=False)
    wlv = nc.declare_dram_parameter("wlv", [K, O_core], bf16, isOutput=False)
    weps = nc.declare_dram_parameter("weps", [K, O_core], bf16, isOutput=False)
    bmu = nc.declare_dram_parameter("bmu", [1, O_core], f32, isOutput=False)
    blv = nc.declare_dram_parameter("blv", [1, O_core], f32, isOutput=False)
    beps = nc.declare_dram_parameter("beps", [1, O_core], f32, isOutput=False)
    out = nc.declare_dram_parameter("out", [B_core, O_core], f32, isOutput=True)

    xt_r = xt.ap().rearrange("(kt p) b -> p kt b", p=128)
    out_r = out.ap().rearrange("(mt p) o -> p mt o", p=128)

    with tile.TileContext(nc) as tc:
        with (
            tc.tile_pool(name="wres", bufs=KT) as wres_pool,
            tc.tile_pool(name="wstage", bufs=2) as wstage_pool,
            tc.tile_pool(name="xblk", bufs=3) as xpool,
            tc.tile_pool(name="osb", bufs=4) as opool,
            tc.tile_pool(name="bias", bufs=1) as bias_pool,
            tc.tile_pool(name="psum", bufs=8, space="PSUM") as ppool,
        ):
            # ---- bias: b = bmu + beps * exp(0.5*blv), replicated to 128 parts
            b_mu_t = bias_pool.tile([1, O_core], f32, tag="bmu")
            b_lv_t = bias_pool.tile([1, O_core], f32, tag="blv")
            b_eps_t = bias_pool.tile([1, O_core], f32, tag="beps")
            nc.sync.dma_start(out=b_mu_t[:], in_=bmu.ap())
            nc.sync.dma_start(out=b_lv_t[:], in_=blv.ap())
            nc.sync.dma_start(out=b_eps_t[:], in_=beps.ap())
            b_sd = bias_pool.tile([1, O_core], f32, tag="bsd")
            nc.scalar.activation(b_sd[:], b_lv_t[:], Exp, scale=0.5)
            nc.vector.tensor_tensor(out=b_sd[:], in0=b_sd[:], in1=b_eps_t[:], op=mult)
            nc.vector.tensor_tensor(out=b_sd[:], in0=b_sd[:], in1=b_mu_t[:], op=add)
            bias_rep = bias_pool.tile([128, O_core], f32, tag="brep")
            nc.gpsimd.partition_broadcast(bias_rep[:], b_sd[:])

            # ---- weight sampling: wres[k] = bf16(mu + eps * exp(0.5*lv))
            wres = []
            for k in range(KT):
                smu = wstage_pool.tile([128, O_core], bf16, tag="smu")
                slv = wstage_pool.tile([128, O_core], bf16, tag="slv")
                seps = wstage_pool.tile([128, O_core], bf16, tag="seps")
                nc.sync.dma_start(out=smu[:], in_=wmu.ap()[k * 128:(k + 1) * 128, :])
                nc.sync.dma_start(out=slv[:], in_=wlv.ap()[k * 128:(k + 1) * 128, :])
                nc.sync.dma_start(out=seps[:], in_=weps.ap()[k * 128:(k + 1) * 128, :])
                w_k = wres_pool.tile([128, O_core], bf16, tag="wres")
                nc.scalar.activation(w_k[:], slv[:], Exp, scale=0.5)
                nc.vector.tensor_tensor(out=w_k[:], in0=w_k[:], in1=seps[:], op=mult)
                nc.vector.tensor_tensor(out=w_k[:], in0=w_k[:], in1=smu[:], op=add)
                wres.append(w_k)

            # ---- matmul: out[b, o] = x @ w.T + bias
            for m in range(MT):
                xtile = xpool.tile([128, KT, 512], bf16, tag="xblk")
                nc.sync.dma_start(out=xtile[:], in_=xt_r[:, :, m * 512:(m + 1) * 512])
                psums = [
                    [ppool.tile([128, 512], f32, tag="ps", name=f"ps_{m}_{ms}_{n}")
                     for n in range(NO)]
                    for ms in range(4)
                ]
                for k in range(KT):
                    for ms in range(4):
                        lhsT = xtile[:, k, ms * 128:(ms + 1) * 128]
                        for n in range(NO):
                            nc.tensor.matmul(
                                psums[ms][n][:],
                                lhsT,
                                wres[k][:, n * 512:(n + 1) * 512],
                                start=(k == 0),
                                stop=(k == KT - 1),
                            )
                for ms in range(4):
                    for n in range(NO):
                        osb = opool.tile([128, 512], f32, tag="osb")
                        nc.vector.tensor_tensor(
                            out=osb[:],
                            in0=psums[ms][n][:],
                            in1=bias_rep[:, n * 512:(n + 1) * 512],
                            op=add,
                        )
                        nc.sync.dma_start(
                            out=out_r[:, m * 4 + ms, n * 512:(n + 1) * 512],
                            in_=osb[:],
                        )

    nc.compile()
    return nc


def _get_program():
    key = (B_CORE, O_CORE, IN_F)
    if key not in _PROGRAM_CACHE:
        _PROGRAM_CACHE[key] = build_program(*key)
    return _PROGRAM_CACHE[key]


def make_in_maps(x, weight_mu, weight_log_var, bias_mu, bias_log_var,
                 weight_eps, bias_eps):
    """Shard + transpose + cast the full inputs into 8 per-core input maps."""
    x = np.asarray(x, dtype=np.float32)
    weight_mu = np.asarray(weight_mu, dtype=np.float32)
    weight_log_var = np.asarray(weight_log_var, dtype=np.float32)
    weight_eps = np.asarray(weight_eps, dtype=np.float32)
    bias_mu = np.asarray(bias_mu, dtype=np.float32).reshape(-1)
    bias_log_var = np.asarray(bias_log_var, dtype=np.float32).reshape(-1)
    bias_eps = np.asarray(bias_eps, dtype=np.float32).reshape(-1)

    xt = np.ascontiguousarray(x.astype(BF16).T)              # [IN_F, BATCH]
    wmu_t = np.ascontiguousarray(weight_mu.astype(BF16).T)   # [IN_F, OUT_F]
    wlv_t = np.ascontiguousarray(weight_log_var.astype(BF16).T)
    weps_t = np.ascontiguousarray(weight_eps.astype(BF16).T)

    in_maps = []
    for c in range(N_CORES):
        bi, oi = divmod(c, O_SHARDS)
        bs = slice(bi * B_CORE, (bi + 1) * B_CORE)
        os_ = slice(oi * O_CORE, (oi + 1) * O_CORE)
        in_maps.append({
            "xt": np.ascontiguousarray(xt[:, bs]),
            "wmu": np.ascontiguousarray(wmu_t[:, os_]),
            "wlv": np.ascontiguousarray(wlv_t[:, os_]),
            "weps": np.ascontiguousarray(weps_t[:, os_]),
            "bmu": np.ascontiguousarray(bias_mu[os_]).reshape(1, O_CORE),
            "blv": np.ascontiguousarray(bias_log_var[os_]).reshape(1, O_CORE),
            "beps": np.ascontiguousarray(bias_eps[os_]).reshape(1, O_CORE),
        })
    return in_maps


def gather_output(results):
    out = np.empty((BATCH, OUT_F), dtype=np.float32)
    for c in range(N_CORES):
        bi, oi = divmod(c, O_SHARDS)
        out[bi * B_CORE:(bi + 1) * B_CORE, oi * O_CORE:(oi + 1) * O_CORE] = \
            results[c]["out"]
    return out


def run_on_hw(in_maps, trace=False):
    from concourse.bass_utils import run_bass_kernel_spmd
    nc = _get_program()
    return run_bass_kernel_spmd(nc, in_maps, list(range(N_CORES)), trace=trace)


_RUNNER = None


def _get_runner():
    """Build (once per process) a cached jit callable: in_maps -> results.

    Mirrors bass2jax.run_bass_via_pjrt's multi-core branch, but keeps the
    jitted executable alive so repeated kernel() calls skip recompilation.
    """
    global _RUNNER
    if _RUNNER is not None:
        return _RUNNER
    import jax
    from jax.sharding import Mesh, PartitionSpec
    try:
        from jax.experimental.shard_map import shard_map
    except ImportError:  # newer jax
        from jax import shard_map
    import concourse.mybir as mybir
    from concourse import bass2jax

    nc = _get_program()
    bass2jax.install_neuronx_cc_hook()
    assert nc.dbg_addr is None and nc.partition_id_tensor is None

    in_names, out_names, out_shapes, out_dtypes = [], [], [], []
    for alloc in nc.m.functions[0].allocations:
        if not isinstance(alloc, mybir.MemoryLocationSet):
            continue
        name = alloc.memorylocations[0].name
        if alloc.kind == "ExternalInput":
            in_names.append(name)
        elif alloc.kind == "ExternalOutput":
            out_names.append(name)
            out_shapes.append(tuple(alloc.tensor_shape))
            out_dtypes.append(mybir.dt.np(alloc.dtype))
    out_avals = [jax.core.ShapedArray(s, d)
                 for s, d in zip(out_shapes, out_dtypes)]
    n_params = len(in_names)
    all_names = tuple(in_names + out_names)

    def _body(*args):
        outs = bass2jax._bass_exec_p.bind(
            *args,
            out_avals=tuple(out_avals),
            in_names=all_names,
            out_names=tuple(out_names),
            lowering_input_output_aliases=(),
            sim_require_finite=True,
            sim_require_nnan=True,
            nc=nc,
        )
        return tuple(outs)

    devices = jax.devices()[:N_CORES]
    assert len(devices) == N_CORES
    mesh = Mesh(np.asarray(devices), ("core",))
    donate = tuple(range(n_params, n_params + len(out_names)))
    sharded = jax.jit(
        shard_map(
            _body, mesh=mesh,
            in_specs=(PartitionSpec("core"),) * (n_params + len(out_names)),
            out_specs=(PartitionSpec("core"),) * len(out_names),
            check_rep=False),
        donate_argnums=donate, keep_unused=True)

    def run(in_maps):
        per_core = [[np.asarray(m[name]) for name in in_names]
                    for m in in_maps]
        concat_in = [
            np.concatenate([per_core[c][i] for c in range(N_CORES)], axis=0)
            for i in range(n_params)
        ]
        zero_outs = [np.zeros((N_CORES * s[0],) + s[1:], d)
                     for s, d in zip(out_shapes, out_dtypes)]
        outs = sharded(*concat_in, *zero_outs)
        results = []
        for c in range(N_CORES):
            m = {}
            for i, name in enumerate(out_names):
                s0 = out_shapes[i][0]
                m[name] = np.asarray(outs[i][c * s0:(c + 1) * s0])
            results.append(m)
        return results

    _RUNNER = run
    return run


def kernel(x, weight_mu, weight_log_var, bias_mu, bias_log_var,
           weight_eps, bias_eps):
    in_maps = make_in_maps(x, weight_mu, weight_log_var, bias_mu,
                           bias_log_var, weight_eps, bias_eps)
    results = _get_runner()(in_maps)
    return gather_output(results)


# revision 6
# speedup vs baseline: 1.0671x; 1.0396x over previous
"""BayesianLinear (reparameterized sampling + linear) on 8 TRN2 NeuronCores.

Math:  w = weight_mu + weight_eps * exp(0.5*weight_log_var)   [OUT_F, IN_F]
       b = bias_mu + bias_eps * exp(0.5*bias_log_var)         [OUT_F]
       out = x @ w.T + b                                      [BATCH, OUT_F]

Sharding: 2-way over BATCH x 4-way over OUT_F (8 cores).  Each core gets
K-major (transposed) bf16 copies of its x / weight shards, samples the
weight on device into an SBUF-resident bf16 [IN_F, O_CORE] tensor, and
streams x through the TensorEngine in bf16 (fp32 PSUM accumulate).
Bias is sampled on device in fp32 and fused into the PSUM->SBUF copy.
"""

import numpy as np
import ml_dtypes

BATCH = 8192
IN_F = 4096
OUT_F = 4096
B_SHARDS = 2
O_SHARDS = 4
N_CORES = B_SHARDS * O_SHARDS

B_CORE = BATCH // B_SHARDS   # 4096
O_CORE = OUT_F // O_SHARDS   # 1024

BF16 = ml_dtypes.bfloat16

_PROGRAM_CACHE = {}


def build_program(B_core=B_CORE, O_core=O_CORE, K=IN_F):
    """Build + compile the per-core Bass/Tile program (same NEFF on all cores).

    DRAM parameters (per core):
      xt   [K, B_core]  bf16   x shard, transposed (K-major)
      wmu  [K, O_core]  bf16   weight_mu shard, transposed
      wlv  [K, O_core]  bf16   weight_log_var shard, transposed
      weps [K, O_core]  bf16   weight_eps shard, transposed
      bmu/blv/beps [1, O_core] f32
      out  [B_core, O_core] f32
    """
    import concourse.mybir as mybir
    import concourse.tile as tile
    from concourse import bacc

    assert K % 128 == 0 and B_core % 512 == 0 and O_core % 512 == 0
    KT = K // 128          # contraction tiles
    MT = B_core // 512     # x blocks (512 batch rows each)
    NO = O_core // 512     # output free-dim chunks

    f32 = mybir.dt.float32
    bf16 = mybir.dt.bfloat16
    Exp = mybir.ActivationFunctionType.Exp
    mult = mybir.AluOpType.mult
    add = mybir.AluOpType.add

    nc = bacc.Bacc("TRN2", target_bir_lowering=False, debug=False)

    xt = nc.declare_dram_parameter("xt", [K, B_core], bf16, isOutput=False)
    wmu = nc.declare_dram_parameter("wmu", [K, O_core], bf16, isOutput=False)
    wlv = nc.declare_dram_parameter("wlv", [K, O_core], bf16, isOutput=False)
    weps = nc.declare_dram_parameter("weps", [K, O_core], bf16, isOutput=False)
    bmu = nc.declare_dram_parameter("bmu", [1, O_core], f32, isOutput=False)
    blv = nc.declare_dram_parameter("blv", [1, O_core], f32, isOutput=False)
    beps = nc.declare_dram_parameter("beps", [1, O_core], f32, isOutput=False)
    out = nc.declare_dram_parameter("out", [B_core, O_core], f32, isOutput=True)

    xt_r = xt.ap().rearrange("(kt p) b -> p kt b", p=128)
    out_r = out.ap().rearrange("(mt p) o -> p mt o", p=128)

    KC = 4                  # k-tiles per weight-stage chunk (batched DMA)
    NC_CHUNKS = KT // KC
    X_BUFS = 2

    with tile.TileContext(nc) as tc:
        with (
            tc.tile_pool(name="wres", bufs=NC_CHUNKS) as wres_pool,
            tc.tile_pool(name="wstage", bufs=2) as wstage_pool,
            tc.tile_pool(name="xblk", bufs=X_BUFS) as xpool,
            tc.tile_pool(name="osb", bufs=4) as opool,
            tc.tile_pool(name="bias", bufs=1) as bias_pool,
            tc.tile_pool(name="psum", bufs=8, space="PSUM") as ppool,
        ):
            # ---- bias: b = bmu + beps * exp(0.5*blv), replicated to 128 parts
            b_mu_t = bias_pool.tile([1, O_core], f32, tag="bmu")
            b_lv_t = bias_pool.tile([1, O_core], f32, tag="blv")
            b_eps_t = bias_pool.tile([1, O_core], f32, tag="beps")
            nc.sync.dma_start(out=b_mu_t[:], in_=bmu.ap())
            nc.sync.dma_start(out=b_lv_t[:], in_=blv.ap())
            nc.sync.dma_start(out=b_eps_t[:], in_=beps.ap())
            b_sd = bias_pool.tile([1, O_core], f32, tag="bsd")
            nc.scalar.activation(b_sd[:], b_lv_t[:], Exp, scale=0.5)
            nc.vector.tensor_tensor(out=b_sd[:], in0=b_sd[:], in1=b_eps_t[:], op=mult)
            nc.vector.tensor_tensor(out=b_sd[:], in0=b_sd[:], in1=b_mu_t[:], op=add)
            bias_rep = bias_pool.tile([128, O_core], f32, tag="brep")
            nc.gpsimd.partition_broadcast(bias_rep[:], b_sd[:])

            # ---- pre-issue x DMA for the first blocks (ahead of the weight
            # stream) so the first matmuls aren't queued behind 25 MB of
            # weight traffic.
            wmu_r = wmu.ap().rearrange("(c p) o -> p c o", p=128)
            wlv_r = wlv.ap().rearrange("(c p) o -> p c o", p=128)
            weps_r = weps.ap().rearrange("(c p) o -> p c o", p=128)

            xtiles = {}
            for m in range(min(X_BUFS, MT)):
                xt_t = xpool.tile([128, KT, 512], bf16, tag="xblk",
                                  name=f"xblk_{m}")
                nc.sync.dma_start(out=xt_t[:],
                                  in_=xt_r[:, :, m * 512:(m + 1) * 512])
                xtiles[m] = xt_t

            # ---- weight sampling: wres = bf16(mu + eps * exp(0.5*lv)),
            # staged KC k-tiles per DMA to amortize descriptor-gen cost.
            wchunks = []
            for c in range(NC_CHUNKS):
                smu = wstage_pool.tile([128, KC, O_core], bf16, tag="smu")
                slv = wstage_pool.tile([128, KC, O_core], bf16, tag="slv")
                seps = wstage_pool.tile([128, KC, O_core], bf16, tag="seps")
                ksl = slice(c * KC, (c + 1) * KC)
                nc.sync.dma_start(out=smu[:], in_=wmu_r[:, ksl, :])
                nc.sync.dma_start(out=slv[:], in_=wlv_r[:, ksl, :])
                nc.sync.dma_start(out=seps[:], in_=weps_r[:, ksl, :])
                w_c = wres_pool.tile([128, KC, O_core], bf16, tag="wres")
                nc.scalar.activation(w_c[:], slv[:], Exp, scale=0.5)
                nc.vector.tensor_tensor(out=w_c[:], in0=w_c[:], in1=seps[:], op=mult)
                nc.vector.tensor_tensor(out=w_c[:], in0=w_c[:], in1=smu[:], op=add)
                wchunks.append(w_c)

            def wres_slice(k, n):
                return wchunks[k // KC][:, k % KC, n * 512:(n + 1) * 512]

            # ---- matmul: out[b, o] = x @ w.T + bias
            for m in range(MT):
                if m in xtiles:
                    xtile = xtiles.pop(m)
                else:
                    xtile = xpool.tile([128, KT, 512], bf16, tag="xblk",
                                       name=f"xblk_{m}")
                    nc.sync.dma_start(out=xtile[:],
                                      in_=xt_r[:, :, m * 512:(m + 1) * 512])
                psums = [
                    [ppool.tile([128, 512], f32, tag="ps", name=f"ps_{m}_{ms}_{n}")
                     for n in range(NO)]
                    for ms in range(4)
                ]
                for k in range(KT):
                    for ms in range(4):
                        lhsT = xtile[:, k, ms * 128:(ms + 1) * 128]
                        for n in range(NO):
                            nc.tensor.matmul(
                                psums[ms][n][:],
                                lhsT,
                                wres_slice(k, n),
                                start=(k == 0),
                                stop=(k == KT - 1),
                            )
                for ms in range(4):
                    for n in range(NO):
                        osb = opool.tile([128, 512], f32, tag="osb")
                        nc.vector.tensor_tensor(
                            out=osb[:],
                            in0=psums[ms][n][:],
                            in1=bias_rep[:, n * 512:(n + 1) * 512],
                            op=add,
                        )
                        nc.sync.dma_start(
                            out=out_r[:, m * 4 + ms, n * 512:(n + 1) * 512],
                            in_=osb[:],
                        )

    nc.compile()
    return nc


def _get_program():
    key = (B_CORE, O_CORE, IN_F)
    if key not in _PROGRAM_CACHE:
        _PROGRAM_CACHE[key] = build_program(*key)
    return _PROGRAM_CACHE[key]


def make_in_maps(x, weight_mu, weight_log_var, bias_mu, bias_log_var,
                 weight_eps, bias_eps):
    """Shard + transpose + cast the full inputs into 8 per-core input maps."""
    x = np.asarray(x, dtype=np.float32)
    weight_mu = np.asarray(weight_mu, dtype=np.float32)
    weight_log_var = np.asarray(weight_log_var, dtype=np.float32)
    weight_eps = np.asarray(weight_eps, dtype=np.float32)
    bias_mu = np.asarray(bias_mu, dtype=np.float32).reshape(-1)
    bias_log_var = np.asarray(bias_log_var, dtype=np.float32).reshape(-1)
    bias_eps = np.asarray(bias_eps, dtype=np.float32).reshape(-1)

    xt = np.ascontiguousarray(x.astype(BF16).T)              # [IN_F, BATCH]
    wmu_t = np.ascontiguousarray(weight_mu.astype(BF16).T)   # [IN_F, OUT_F]
    wlv_t = np.ascontiguousarray(weight_log_var.astype(BF16).T)
    weps_t = np.ascontiguousarray(weight_eps.astype(BF16).T)

    in_maps = []
    for c in range(N_CORES):
        bi, oi = divmod(c, O_SHARDS)
        bs = slice(bi * B_CORE, (bi + 1) * B_CORE)
        os_ = slice(oi * O_CORE, (oi + 1) * O_CORE)
        in_maps.append({
            "xt": np.ascontiguousarray(xt[:, bs]),
            "wmu": np.ascontiguousarray(wmu_t[:, os_]),
            "wlv": np.ascontiguousarray(wlv_t[:, os_]),
            "weps": np.ascontiguousarray(weps_t[:, os_]),
            "bmu": np.ascontiguousarray(bias_mu[os_]).reshape(1, O_CORE),
            "blv": np.ascontiguousarray(bias_log_var[os_]).reshape(1, O_CORE),
            "beps": np.ascontiguousarray(bias_eps[os_]).reshape(1, O_CORE),
        })
    return in_maps


def gather_output(results):
    out = np.empty((BATCH, OUT_F), dtype=np.float32)
    for c in range(N_CORES):
        bi, oi = divmod(c, O_SHARDS)
        out[bi * B_CORE:(bi + 1) * B_CORE, oi * O_CORE:(oi + 1) * O_CORE] = \
            results[c]["out"]
    return out


def run_on_hw(in_maps, trace=False):
    from concourse.bass_utils import run_bass_kernel_spmd
    nc = _get_program()
    return run_bass_kernel_spmd(nc, in_maps, list(range(N_CORES)), trace=trace)


_RUNNER = None


def _get_runner():
    """Build (once per process) a cached jit callable: in_maps -> results.

    Mirrors bass2jax.run_bass_via_pjrt's multi-core branch, but keeps the
    jitted executable alive so repeated kernel() calls skip recompilation.
    """
    global _RUNNER
    if _RUNNER is not None:
        return _RUNNER
    import jax
    from jax.sharding import Mesh, PartitionSpec
    try:
        from jax.experimental.shard_map import shard_map
    except ImportError:  # newer jax
        from jax import shard_map
    import concourse.mybir as mybir
    from concourse import bass2jax

    nc = _get_program()
    bass2jax.install_neuronx_cc_hook()
    assert nc.dbg_addr is None and nc.partition_id_tensor is None

    in_names, out_names, out_shapes, out_dtypes = [], [], [], []
    for alloc in nc.m.functions[0].allocations:
        if not isinstance(alloc, mybir.MemoryLocationSet):
            continue
        name = alloc.memorylocations[0].name
        if alloc.kind == "ExternalInput":
            in_names.append(name)
        elif alloc.kind == "ExternalOutput":
            out_names.append(name)
            out_shapes.append(tuple(alloc.tensor_shape))
            out_dtypes.append(mybir.dt.np(alloc.dtype))
    out_avals = [jax.core.ShapedArray(s, d)
                 for s, d in zip(out_shapes, out_dtypes)]
    n_params = len(in_names)
    all_names = tuple(in_names + out_names)

    def _body(*args):
        outs = bass2jax._bass_exec_p.bind(
            *args,
            out_avals=tuple(out_avals),
            in_names=all_names,
            out_names=tuple(out_names),
            lowering_input_output_aliases=(),
            sim_require_finite=True,
            sim_require_nnan=True,
            nc=nc,
        )
        return tuple(outs)

    devices = jax.devices()[:N_CORES]
    assert len(devices) == N_CORES
    mesh = Mesh(np.asarray(devices), ("core",))
    donate = tuple(range(n_params, n_params + len(out_names)))
    sharded = jax.jit(
        shard_map(
            _body, mesh=mesh,
            in_specs=(PartitionSpec("core"),) * (n_params + len(out_names)),
            out_specs=(PartitionSpec("core"),) * len(out_names),
            check_rep=False),
        donate_argnums=donate, keep_unused=True)

    def run(in_maps):
        per_core = [[np.asarray(m[name]) for name in in_names]
                    for m in in_maps]
        concat_in = [
            np.concatenate([per_core[c][i] for c in range(N_CORES)], axis=0)
            for i in range(n_params)
        ]
        zero_outs = [np.zeros((N_CORES * s[0],) + s[1:], d)
                     for s, d in zip(out_shapes, out_dtypes)]
        outs = sharded(*concat_in, *zero_outs)
        results = []
        for c in range(N_CORES):
            m = {}
            for i, name in enumerate(out_names):
                s0 = out_shapes[i][0]
                m[name] = np.asarray(outs[i][c * s0:(c + 1) * s0])
            results.append(m)
        return results

    _RUNNER = run
    return run


def kernel(x, weight_mu, weight_log_var, bias_mu, bias_log_var,
           weight_eps, bias_eps):
    in_maps = make_in_maps(x, weight_mu, weight_log_var, bias_mu,
                           bias_log_var, weight_eps, bias_eps)
    results = _get_runner()(in_maps)
    return gather_output(results)


# revision 8
# speedup vs baseline: 1.0823x; 1.0142x over previous
"""BayesianLinear (reparameterized sampling + linear) on 8 TRN2 NeuronCores.

Math:  w = weight_mu + weight_eps * exp(0.5*weight_log_var)   [OUT_F, IN_F]
       b = bias_mu + bias_eps * exp(0.5*bias_log_var)         [OUT_F]
       out = x @ w.T + b                                      [BATCH, OUT_F]

Sharding: 2-way over BATCH x 4-way over OUT_F (8 cores).  Each core gets
K-major (transposed) bf16 copies of its x / weight shards, samples the
weight on device into an SBUF-resident bf16 [IN_F, O_CORE] tensor, and
streams x through the TensorEngine in bf16 (fp32 PSUM accumulate).
Bias is sampled on device in fp32 and fused into the PSUM->SBUF copy.
"""

import numpy as np
import ml_dtypes

BATCH = 8192
IN_F = 4096
OUT_F = 4096
B_SHARDS = 2
O_SHARDS = 4
N_CORES = B_SHARDS * O_SHARDS

B_CORE = BATCH // B_SHARDS   # 4096
O_CORE = OUT_F // O_SHARDS   # 1024

BF16 = ml_dtypes.bfloat16

_PROGRAM_CACHE = {}


def build_program(B_core=B_CORE, O_core=O_CORE, K=IN_F):
    """Build + compile the per-core Bass/Tile program (same NEFF on all cores).

    DRAM parameters (per core):
      xt   [K, B_core]  bf16   x shard, transposed (K-major)
      wmu  [K, O_core]  bf16   weight_mu shard, transposed
      wlv  [K, O_core]  bf16   weight_log_var shard, transposed
      weps [K, O_core]  bf16   weight_eps shard, transposed
      bmu/blv/beps [1, O_core] f32
      out  [B_core, O_core] f32
    """
    import concourse.mybir as mybir
    import concourse.tile as tile
    from concourse import bacc

    assert K % 128 == 0 and B_core % 512 == 0 and O_core % 512 == 0
    KT = K // 128          # contraction tiles
    MT = B_core // 512     # x blocks (512 batch rows each)
    NO = O_core // 512     # output free-dim chunks

    f32 = mybir.dt.float32
    bf16 = mybir.dt.bfloat16
    Exp = mybir.ActivationFunctionType.Exp
    mult = mybir.AluOpType.mult
    add = mybir.AluOpType.add

    nc = bacc.Bacc("TRN2", target_bir_lowering=False, debug=False)

    xt = nc.declare_dram_parameter("xt", [K, B_core], bf16, isOutput=False)
    wmu = nc.declare_dram_parameter("wmu", [K, O_core], bf16, isOutput=False)
    wlv = nc.declare_dram_parameter("wlv", [K, O_core], bf16, isOutput=False)
    weps = nc.declare_dram_parameter("weps", [K, O_core], bf16, isOutput=False)
    bmu = nc.declare_dram_parameter("bmu", [1, O_core], f32, isOutput=False)
    blv = nc.declare_dram_parameter("blv", [1, O_core], f32, isOutput=False)
    beps = nc.declare_dram_parameter("beps", [1, O_core], f32, isOutput=False)
    out = nc.declare_dram_parameter("out", [B_core, O_core], f32, isOutput=True)

    xt_r = xt.ap().rearrange("(kt p) b -> p kt b", p=128)
    out_r = out.ap().rearrange("(mt p) o -> p mt o", p=128)

    KC = 4                  # k-tiles per weight-stage chunk (batched DMA)
    NC_CHUNKS = KT // KC
    X_BUFS = 2

    with tile.TileContext(nc) as tc:
        with (
            tc.tile_pool(name="wres", bufs=NC_CHUNKS) as wres_pool,
            tc.tile_pool(name="wstage", bufs=2) as wstage_pool,
            tc.tile_pool(name="xblk", bufs=X_BUFS * NC_CHUNKS) as xpool,
            tc.tile_pool(name="osb", bufs=4) as opool,
            tc.tile_pool(name="bias", bufs=1) as bias_pool,
            tc.tile_pool(name="psum", bufs=8, space="PSUM") as ppool,
        ):
            # ---- bias: b = bmu + beps * exp(0.5*blv), replicated to 128 parts
            b_mu_t = bias_pool.tile([1, O_core], f32, tag="bmu")
            b_lv_t = bias_pool.tile([1, O_core], f32, tag="blv")
            b_eps_t = bias_pool.tile([1, O_core], f32, tag="beps")
            nc.sync.dma_start(out=b_mu_t[:], in_=bmu.ap())
            nc.sync.dma_start(out=b_lv_t[:], in_=blv.ap())
            nc.sync.dma_start(out=b_eps_t[:], in_=beps.ap())
            b_sd = bias_pool.tile([1, O_core], f32, tag="bsd")
            nc.scalar.activation(b_sd[:], b_lv_t[:], Exp, scale=0.5)
            nc.vector.tensor_tensor(out=b_sd[:], in0=b_sd[:], in1=b_eps_t[:], op=mult)
            nc.vector.tensor_tensor(out=b_sd[:], in0=b_sd[:], in1=b_mu_t[:], op=add)
            bias_rep = bias_pool.tile([128, O_core], f32, tag="brep")
            nc.gpsimd.partition_broadcast(bias_rep[:], b_sd[:])

            # x is loaded per (block, k-chunk) so the first matmuls only wait
            # for one 0.5 MB x chunk + one weight chunk, not a whole block.
            wmu_r = wmu.ap().rearrange("(c p) o -> p c o", p=128)
            wlv_r = wlv.ap().rearrange("(c p) o -> p c o", p=128)
            weps_r = weps.ap().rearrange("(c p) o -> p c o", p=128)
            xt_c = xt.ap().rearrange("(c kc p) b -> p c kc b", p=128, kc=KC)

            def load_x_chunk(m, c):
                t = xpool.tile([128, KC, 512], bf16, tag="xblk",
                               name=f"xblk_{m}_{c}")
                nc.sync.dma_start(out=t[:],
                                  in_=xt_c[:, c, :, m * 512:(m + 1) * 512])
                return t

            def load_w_chunk(c):
                smu = wstage_pool.tile([128, KC, O_core], bf16, tag="smu")
                slv = wstage_pool.tile([128, KC, O_core], bf16, tag="slv")
                seps = wstage_pool.tile([128, KC, O_core], bf16, tag="seps")
                ksl = slice(c * KC, (c + 1) * KC)
                nc.sync.dma_start(out=smu[:], in_=wmu_r[:, ksl, :])
                nc.sync.dma_start(out=slv[:], in_=wlv_r[:, ksl, :])
                nc.sync.dma_start(out=seps[:], in_=weps_r[:, ksl, :])
                w_c = wres_pool.tile([128, KC, O_core], bf16, tag="wres")
                nc.scalar.activation(w_c[:], slv[:], Exp, scale=0.5)
                nc.vector.tensor_tensor(out=w_c[:], in0=w_c[:], in1=seps[:], op=mult)
                nc.vector.tensor_tensor(out=w_c[:], in0=w_c[:], in1=smu[:], op=add)
                return w_c

            # Interleave m0's x chunks with the weight chunks in the DMA
            # queue: MM group k-chunk c needs exactly (x0[c], w[c]).
            xtiles = {}
            wchunks = []
            xtiles[0] = []
            for c in range(NC_CHUNKS):
                xtiles[0].append(load_x_chunk(0, c))
                wchunks.append(load_w_chunk(c))
            if MT > 1:
                xtiles[1] = [load_x_chunk(1, c) for c in range(NC_CHUNKS)]

            def wres_slice(k, n):
                return wchunks[k // KC][:, k % KC, n * 512:(n + 1) * 512]

            # ---- matmul: out[b, o] = x @ w.T + bias
            def emit_group(m, psum, xchunks, ms, n, k_range):
                for k in k_range:
                    nc.tensor.matmul(
                        psum[:],
                        xchunks[k // KC][:, k % KC, ms * 128:(ms + 1) * 128],
                        wres_slice(k, n),
                        start=(k == 0),
                        stop=(k == KT - 1),
                    )

            def copy_out(m, psum, ms, n):
                osb = opool.tile([128, 512], f32, tag="osb",
                                 name=f"osb_{m}_{ms}_{n}")
                nc.vector.tensor_tensor(
                    out=osb[:], in0=psum[:],
                    in1=bias_rep[:, n * 512:(n + 1) * 512], op=add)
                nc.sync.dma_start(
                    out=out_r[:, m * 4 + ms, n * 512:(n + 1) * 512],
                    in_=osb[:])

            for m in range(MT):
                if m in xtiles:
                    xchunks = xtiles.pop(m)
                else:
                    xchunks = [load_x_chunk(m, c) for c in range(NC_CHUNKS)]
                if m == 0:
                    # k-outer: all 8 psum groups accumulate in lockstep with
                    # the arriving weight stream.
                    psums = [
                        [ppool.tile([128, 512], f32, tag="ps",
                                    name=f"ps_{m}_{ms}_{n}") for n in range(NO)]
                        for ms in range(4)
                    ]
                    for k in range(KT):
                        for ms in range(4):
                            for n in range(NO):
                                emit_group(m, psums[ms][n], xchunks, ms, n, [k])
                    for ms in range(4):
                        for n in range(NO):
                            copy_out(m, psums[ms][n], ms, n)
                else:
                    # k-inner: each group finishes + copies out immediately,
                    # overlapping the next group's matmuls.
                    for ms in range(4):
                        for n in range(NO):
                            psum = ppool.tile([128, 512], f32, tag="ps",
                                              name=f"ps_{m}_{ms}_{n}")
                            emit_group(m, psum, xchunks, ms, n, range(KT))
                            copy_out(m, psum, ms, n)

    nc.compile()
    return nc


def _get_program():
    key = (B_CORE, O_CORE, IN_F)
    if key not in _PROGRAM_CACHE:
        _PROGRAM_CACHE[key] = build_program(*key)
    return _PROGRAM_CACHE[key]


def make_in_maps(x, weight_mu, weight_log_var, bias_mu, bias_log_var,
                 weight_eps, bias_eps):
    """Shard + transpose + cast the full inputs into 8 per-core input maps."""
    x = np.asarray(x, dtype=np.float32)
    weight_mu = np.asarray(weight_mu, dtype=np.float32)
    weight_log_var = np.asarray(weight_log_var, dtype=np.float32)
    weight_eps = np.asarray(weight_eps, dtype=np.float32)
    bias_mu = np.asarray(bias_mu, dtype=np.float32).reshape(-1)
    bias_log_var = np.asarray(bias_log_var, dtype=np.float32).reshape(-1)
    bias_eps = np.asarray(bias_eps, dtype=np.float32).reshape(-1)

    xt = np.ascontiguousarray(x.astype(BF16).T)              # [IN_F, BATCH]
    wmu_t = np.ascontiguousarray(weight_mu.astype(BF16).T)   # [IN_F, OUT_F]
    wlv_t = np.ascontiguousarray(weight_log_var.astype(BF16).T)
    weps_t = np.ascontiguousarray(weight_eps.astype(BF16).T)

    in_maps = []
    for c in range(N_CORES):
        bi, oi = divmod(c, O_SHARDS)
        bs = slice(bi * B_CORE, (bi + 1) * B_CORE)
        os_ = slice(oi * O_CORE, (oi + 1) * O_CORE)
        in_maps.append({
            "xt": np.ascontiguousarray(xt[:, bs]),
            "wmu": np.ascontiguousarray(wmu_t[:, os_]),
            "wlv": np.ascontiguousarray(wlv_t[:, os_]),
            "weps": np.ascontiguousarray(weps_t[:, os_]),
            "bmu": np.ascontiguousarray(bias_mu[os_]).reshape(1, O_CORE),
            "blv": np.ascontiguousarray(bias_log_var[os_]).reshape(1, O_CORE),
            "beps": np.ascontiguousarray(bias_eps[os_]).reshape(1, O_CORE),
        })
    return in_maps


def gather_output(results):
    out = np.empty((BATCH, OUT_F), dtype=np.float32)
    for c in range(N_CORES):
        bi, oi = divmod(c, O_SHARDS)
        out[bi * B_CORE:(bi + 1) * B_CORE, oi * O_CORE:(oi + 1) * O_CORE] = \
            results[c]["out"]
    return out


def run_on_hw(in_maps, trace=False):
    from concourse.bass_utils import run_bass_kernel_spmd
    nc = _get_program()
    return run_bass_kernel_spmd(nc, in_maps, list(range(N_CORES)), trace=trace)


_RUNNER = None


def _get_runner():
    """Build (once per process) a cached jit callable: in_maps -> results.

    Mirrors bass2jax.run_bass_via_pjrt's multi-core branch, but keeps the
    jitted executable alive so repeated kernel() calls skip recompilation.
    """
    global _RUNNER
    if _RUNNER is not None:
        return _RUNNER
    import jax
    from jax.sharding import Mesh, PartitionSpec
    try:
        from jax.experimental.shard_map import shard_map
    except ImportError:  # newer jax
        from jax import shard_map
    import concourse.mybir as mybir
    from concourse import bass2jax

    nc = _get_program()
    bass2jax.install_neuronx_cc_hook()
    assert nc.dbg_addr is None and nc.partition_id_tensor is None

    in_names, out_names, out_shapes, out_dtypes = [], [], [], []
    for alloc in nc.m.functions[0].allocations:
        if not isinstance(alloc, mybir.MemoryLocationSet):
            continue
        name = alloc.memorylocations[0].name
        if alloc.kind == "ExternalInput":
            in_names.append(name)
        elif alloc.kind == "ExternalOutput":
            out_names.append(name)
            out_shapes.append(tuple(alloc.tensor_shape))
            out_dtypes.append(mybir.dt.np(alloc.dtype))
    out_avals = [jax.core.ShapedArray(s, d)
                 for s, d in zip(out_shapes, out_dtypes)]
    n_params = len(in_names)
    all_names = tuple(in_names + out_names)

    def _body(*args):
        outs = bass2jax._bass_exec_p.bind(
            *args,
            out_avals=tuple(out_avals),
            in_names=all_names,
            out_names=tuple(out_names),
            lowering_input_output_aliases=(),
            sim_require_finite=True,
            sim_require_nnan=True,
            nc=nc,
        )
        return tuple(outs)

    devices = jax.devices()[:N_CORES]
    assert len(devices) == N_CORES
    mesh = Mesh(np.asarray(devices), ("core",))
    donate = tuple(range(n_params, n_params + len(out_names)))
    sharded = jax.jit(
        shard_map(
            _body, mesh=mesh,
            in_specs=(PartitionSpec("core"),) * (n_params + len(out_names)),
            out_specs=(PartitionSpec("core"),) * len(out_names),
            check_rep=False),
        donate_argnums=donate, keep_unused=True)

    def run(in_maps):
        per_core = [[np.asarray(m[name]) for name in in_names]
                    for m in in_maps]
        concat_in = [
            np.concatenate([per_core[c][i] for c in range(N_CORES)], axis=0)
            for i in range(n_params)
        ]
        zero_outs = [np.zeros((N_CORES * s[0],) + s[1:], d)
                     for s, d in zip(out_shapes, out_dtypes)]
        outs = sharded(*concat_in, *zero_outs)
        results = []
        for c in range(N_CORES):
            m = {}
            for i, name in enumerate(out_names):
                s0 = out_shapes[i][0]
                m[name] = np.asarray(outs[i][c * s0:(c + 1) * s0])
            results.append(m)
        return results

    _RUNNER = run
    return run


def kernel(x, weight_mu, weight_log_var, bias_mu, bias_log_var,
           weight_eps, bias_eps):
    in_maps = make_in_maps(x, weight_mu, weight_log_var, bias_mu,
                           bias_log_var, weight_eps, bias_eps)
    results = _get_runner()(in_maps)
    return gather_output(results)


# revision 15
# speedup vs baseline: 1.0944x; 1.0112x over previous
"""BayesianLinear (reparameterized sampling + linear) on 8 TRN2 NeuronCores.

Math:  w = weight_mu + weight_eps * exp(0.5*weight_log_var)   [OUT_F, IN_F]
       b = bias_mu + bias_eps * exp(0.5*bias_log_var)         [OUT_F]
       out = x @ w.T + b                                      [BATCH, OUT_F]

Sharding: 2-way over BATCH x 4-way over OUT_F (8 cores).  Each core gets
K-major (transposed) bf16 copies of its x / weight shards, samples the
weight on device into an SBUF-resident bf16 [IN_F, O_CORE] tensor, and
streams x through the TensorEngine in bf16 (fp32 PSUM accumulate).
Bias is sampled on device in fp32 and fused into the PSUM->SBUF copy.
"""

import numpy as np
import ml_dtypes

BATCH = 8192
IN_F = 4096
OUT_F = 4096
B_SHARDS = 2
O_SHARDS = 4
N_CORES = B_SHARDS * O_SHARDS

B_CORE = BATCH // B_SHARDS   # 4096
O_CORE = OUT_F // O_SHARDS   # 1024

BF16 = ml_dtypes.bfloat16

_PROGRAM_CACHE = {}


def build_program(B_core=B_CORE, O_core=O_CORE, K=IN_F):
    """Build + compile the per-core Bass/Tile program (same NEFF on all cores).

    DRAM parameters (per core):
      xt   [K, B_core]  bf16   x shard, transposed (K-major)
      wmu  [K, O_core]  bf16   weight_mu shard, transposed
      wlv  [K, O_core]  bf16   weight_log_var shard, transposed
      weps [K, O_core]  bf16   weight_eps shard, transposed
      bmu/blv/beps [1, O_core] f32
      out  [B_core, O_core] f32
    """
    import concourse.mybir as mybir
    import concourse.tile as tile
    from concourse import bacc

    assert K % 128 == 0 and B_core % 512 == 0 and O_core % 512 == 0
    KT = K // 128          # contraction tiles
    MT = B_core // 512     # x blocks (512 batch rows each)
    NO = O_core // 512     # output free-dim chunks

    f32 = mybir.dt.float32
    bf16 = mybir.dt.bfloat16
    Exp = mybir.ActivationFunctionType.Exp
    mult = mybir.AluOpType.mult
    add = mybir.AluOpType.add

    nc = bacc.Bacc("TRN2", target_bir_lowering=False, debug=False)

    xt = nc.declare_dram_parameter("xt", [K, B_core], bf16, isOutput=False)
    wmu = nc.declare_dram_parameter("wmu", [K, O_core], bf16, isOutput=False)
    wlv = nc.declare_dram_parameter("wlv", [K, O_core], bf16, isOutput=False)
    weps = nc.declare_dram_parameter("weps", [K, O_core], bf16, isOutput=False)
    bmu = nc.declare_dram_parameter("bmu", [1, O_core], f32, isOutput=False)
    blv = nc.declare_dram_parameter("blv", [1, O_core], f32, isOutput=False)
    beps = nc.declare_dram_parameter("beps", [1, O_core], f32, isOutput=False)
    out = nc.declare_dram_parameter("out", [B_core, O_core], f32, isOutput=True)

    xt_r = xt.ap().rearrange("(kt p) b -> p kt b", p=128)
    out_r = out.ap().rearrange("(mt p) o -> p mt o", p=128)

    KC = 4                  # k-tiles per x chunk (batched DMA)
    NC_CHUNKS = KT // KC
    X_BUFS = 2
    # Weight-stage chunk sizes (in k-tiles). Small leading chunks get the
    # first matmuls started ~15 us earlier; 4-tile chunks amortize the
    # ~1 us/dma_start enqueue cost for the bulk of the stream.
    WSIZES = [1, 1, 2] + [4] * ((KT - 4) // 4) if KT >= 8 else [1] * KT
    assert sum(WSIZES) == KT
    WSTARTS = [sum(WSIZES[:i]) for i in range(len(WSIZES))]
    K2C = []
    for ci, (s, st) in enumerate(zip(WSIZES, WSTARTS)):
        K2C += [(ci, k - st) for k in range(st, st + s)]

    with tile.TileContext(nc) as tc:
        with (
            tc.tile_pool(name="wres", bufs=1) as wres_pool,
            tc.tile_pool(name="wstage", bufs=2) as wstage_pool,
            tc.tile_pool(name="xblk", bufs=X_BUFS * NC_CHUNKS) as xpool,
            tc.tile_pool(name="osb", bufs=3) as opool,
            tc.tile_pool(name="bias", bufs=1) as bias_pool,
            tc.tile_pool(name="psum", bufs=8, space="PSUM") as ppool,
        ):
            # ---- bias: b = bmu + beps * exp(0.5*blv), replicated to 128 parts
            bstage = bias_pool.tile([1, 3 * O_core], f32, tag="bstage")
            b_lv_t = bstage[:, 0:O_core]
            b_eps_t = bstage[:, O_core:2 * O_core]
            b_mu_t = bstage[:, 2 * O_core:3 * O_core]
            nc.sync.dma_start(out=b_lv_t, in_=blv.ap())
            nc.sync.dma_start(out=b_eps_t, in_=beps.ap())
            nc.sync.dma_start(out=b_mu_t, in_=bmu.ap())
            nc.scalar.activation(b_lv_t, b_lv_t, Exp, scale=0.5)
            nc.vector.tensor_tensor(out=b_lv_t, in0=b_lv_t, in1=b_eps_t, op=mult)
            nc.vector.tensor_tensor(out=b_lv_t, in0=b_lv_t, in1=b_mu_t, op=add)
            bias_rep = bias_pool.tile([128, O_core], f32, tag="brep")
            nc.gpsimd.partition_broadcast(bias_rep[:], b_lv_t)

            # x is loaded per (block, k-chunk) so the first matmuls only wait
            # for one 0.5 MB x chunk + one weight chunk, not a whole block.
            wmu_r = wmu.ap().rearrange("(c p) o -> p c o", p=128)
            wlv_r = wlv.ap().rearrange("(c p) o -> p c o", p=128)
            weps_r = weps.ap().rearrange("(c p) o -> p c o", p=128)
            xt_c = xt.ap().rearrange("(c kc p) b -> p c kc b", p=128, kc=KC)

            def load_x_chunk(m, c):
                t = xpool.tile([128, KC, 512], bf16, tag="xblk",
                               name=f"xblk_{m}_{c}")
                nc.sync.dma_start(out=t[:],
                                  in_=xt_c[:, c, :, m * 512:(m + 1) * 512])
                return t

            def load_w_chunk(ci):
                size, st = WSIZES[ci], WSTARTS[ci]
                ksl = slice(st, st + size)
                # DMA order = critical-path order: exp needs lv first,
                # then mult needs eps, add needs mu last.
                slv = wstage_pool.tile([128, size, O_core], bf16, tag="slv",
                                       name=f"slv_{ci}")
                seps = wstage_pool.tile([128, size, O_core], bf16, tag="seps",
                                        name=f"seps_{ci}")
                smu = wstage_pool.tile([128, size, O_core], bf16, tag="smu",
                                       name=f"smu_{ci}")
                nc.sync.dma_start(out=slv[:], in_=wlv_r[:, ksl, :])
                nc.sync.dma_start(out=seps[:], in_=weps_r[:, ksl, :])
                nc.sync.dma_start(out=smu[:], in_=wmu_r[:, ksl, :])
                small = size < max(WSIZES)
                w_c = wres_pool.tile(
                    [128, size, O_core], bf16,
                    tag="wres_s" if small else "wres",
                    bufs=(sum(1 for s in WSIZES if s < max(WSIZES)) if small
                          else sum(1 for s in WSIZES if s == max(WSIZES))),
                    name=f"wres_{ci}")
                nc.scalar.activation(w_c[:], slv[:], Exp, scale=0.5)
                nc.vector.tensor_tensor(out=w_c[:], in0=w_c[:], in1=seps[:], op=mult)
                nc.vector.tensor_tensor(out=w_c[:], in0=w_c[:], in1=smu[:], op=add)
                return w_c

            # Interleave m0's x chunks with the weight chunks in the DMA
            # queue, aligned so MMs at k have both x0[k] and w[k] as early
            # as possible.
            xtiles = {0: []}
            wchunks = []
            next_w = 0
            for c in range(NC_CHUNKS):
                xtiles[0].append(load_x_chunk(0, c))
                want_k = (c + 1) * KC  # w coverage needed for x chunks 0..c
                while next_w < len(WSIZES) and WSTARTS[next_w] < want_k:
                    wchunks.append(load_w_chunk(next_w))
                    next_w += 1
            while next_w < len(WSIZES):
                wchunks.append(load_w_chunk(next_w))
                next_w += 1
            if MT > 1:
                xtiles[1] = [load_x_chunk(1, c) for c in range(NC_CHUNKS)]

            def wres_slice(k, n):
                ci, off = K2C[k]
                return wchunks[ci][:, off, n * 512:(n + 1) * 512]

            # ---- matmul: out[b, o] = x @ w.T + bias
            def emit_group(m, psum, xchunks, ms, n, k_range):
                for k in k_range:
                    nc.tensor.matmul(
                        psum[:],
                        xchunks[k // KC][:, k % KC, ms * 128:(ms + 1) * 128],
                        wres_slice(k, n),
                        start=(k == 0),
                        stop=(k == KT - 1),
                    )

            def copy_out(m, psum, ms, n):
                osb = opool.tile([128, 512], f32, tag="osb",
                                 name=f"osb_{m}_{ms}_{n}")
                nc.vector.tensor_tensor(
                    out=osb[:], in0=psum[:],
                    in1=bias_rep[:, n * 512:(n + 1) * 512], op=add)
                nc.sync.dma_start(
                    out=out_r[:, m * 4 + ms, n * 512:(n + 1) * 512],
                    in_=osb[:])

            for m in range(MT):
                if m in xtiles:
                    xchunks = xtiles.pop(m)
                else:
                    xchunks = [load_x_chunk(m, c) for c in range(NC_CHUNKS)]
                if m == 0:
                    # k-outer: all 8 psum groups accumulate in lockstep with
                    # the arriving weight stream.
                    psums = [
                        [ppool.tile([128, 512], f32, tag="ps",
                                    name=f"ps_{m}_{ms}_{n}") for n in range(NO)]
                        for ms in range(4)
                    ]
                    for k in range(KT):
                        for ms in range(4):
                            for n in range(NO):
                                emit_group(m, psums[ms][n], xchunks, ms, n, [k])
                    for ms in range(4):
                        for n in range(NO):
                            copy_out(m, psums[ms][n], ms, n)
                else:
                    # k-inner: each group finishes + copies out immediately,
                    # overlapping the next group's matmuls.
                    for ms in range(4):
                        for n in range(NO):
                            psum = ppool.tile([128, 512], f32, tag="ps",
                                              name=f"ps_{m}_{ms}_{n}")
                            emit_group(m, psum, xchunks, ms, n, range(KT))
                            copy_out(m, psum, ms, n)

    nc.compile()
    return nc


def _get_program():
    key = (B_CORE, O_CORE, IN_F)
    if key not in _PROGRAM_CACHE:
        _PROGRAM_CACHE[key] = build_program(*key)
    return _PROGRAM_CACHE[key]


def make_in_maps(x, weight_mu, weight_log_var, bias_mu, bias_log_var,
                 weight_eps, bias_eps):
    """Shard + transpose + cast the full inputs into 8 per-core input maps."""
    x = np.asarray(x, dtype=np.float32)
    weight_mu = np.asarray(weight_mu, dtype=np.float32)
    weight_log_var = np.asarray(weight_log_var, dtype=np.float32)
    weight_eps = np.asarray(weight_eps, dtype=np.float32)
    bias_mu = np.asarray(bias_mu, dtype=np.float32).reshape(-1)
    bias_log_var = np.asarray(bias_log_var, dtype=np.float32).reshape(-1)
    bias_eps = np.asarray(bias_eps, dtype=np.float32).reshape(-1)

    xt = np.ascontiguousarray(x.astype(BF16).T)              # [IN_F, BATCH]
    wmu_t = np.ascontiguousarray(weight_mu.astype(BF16).T)   # [IN_F, OUT_F]
    wlv_t = np.ascontiguousarray(weight_log_var.astype(BF16).T)
    weps_t = np.ascontiguousarray(weight_eps.astype(BF16).T)

    in_maps = []
    for c in range(N_CORES):
        bi, oi = divmod(c, O_SHARDS)
        bs = slice(bi * B_CORE, (bi + 1) * B_CORE)
        os_ = slice(oi * O_CORE, (oi + 1) * O_CORE)
        in_maps.append({
            "xt": np.ascontiguousarray(xt[:, bs]),
            "wmu": np.ascontiguousarray(wmu_t[:, os_]),
            "wlv": np.ascontiguousarray(wlv_t[:, os_]),
            "weps": np.ascontiguousarray(weps_t[:, os_]),
            "bmu": np.ascontiguousarray(bias_mu[os_]).reshape(1, O_CORE),
            "blv": np.ascontiguousarray(bias_log_var[os_]).reshape(1, O_CORE),
            "beps": np.ascontiguousarray(bias_eps[os_]).reshape(1, O_CORE),
        })
    return in_maps


def gather_output(results):
    out = np.empty((BATCH, OUT_F), dtype=np.float32)
    for c in range(N_CORES):
        bi, oi = divmod(c, O_SHARDS)
        out[bi * B_CORE:(bi + 1) * B_CORE, oi * O_CORE:(oi + 1) * O_CORE] = \
            results[c]["out"]
    return out


def run_on_hw(in_maps, trace=False):
    from concourse.bass_utils import run_bass_kernel_spmd
    nc = _get_program()
    return run_bass_kernel_spmd(nc, in_maps, list(range(N_CORES)), trace=trace)


_RUNNER = None


def _get_runner():
    """Build (once per process) a cached jit callable: in_maps -> results.

    Mirrors bass2jax.run_bass_via_pjrt's multi-core branch, but keeps the
    jitted executable alive so repeated kernel() calls skip recompilation.
    """
    global _RUNNER
    if _RUNNER is not None:
        return _RUNNER
    import jax
    from jax.sharding import Mesh, PartitionSpec
    try:
        from jax.experimental.shard_map import shard_map
    except ImportError:  # newer jax
        from jax import shard_map
    import concourse.mybir as mybir
    from concourse import bass2jax

    nc = _get_program()
    bass2jax.install_neuronx_cc_hook()
    assert nc.dbg_addr is None and nc.partition_id_tensor is None

    in_names, out_names, out_shapes, out_dtypes = [], [], [], []
    for alloc in nc.m.functions[0].allocations:
        if not isinstance(alloc, mybir.MemoryLocationSet):
            continue
        name = alloc.memorylocations[0].name
        if alloc.kind == "ExternalInput":
            in_names.append(name)
        elif alloc.kind == "ExternalOutput":
            out_names.append(name)
            out_shapes.append(tuple(alloc.tensor_shape))
            out_dtypes.append(mybir.dt.np(alloc.dtype))
    out_avals = [jax.core.ShapedArray(s, d)
                 for s, d in zip(out_shapes, out_dtypes)]
    n_params = len(in_names)
    all_names = tuple(in_names + out_names)

    def _body(*args):
        outs = bass2jax._bass_exec_p.bind(
            *args,
            out_avals=tuple(out_avals),
            in_names=all_names,
            out_names=tuple(out_names),
            lowering_input_output_aliases=(),
            sim_require_finite=True,
            sim_require_nnan=True,
            nc=nc,
        )
        return tuple(outs)

    devices = jax.devices()[:N_CORES]
    assert len(devices) == N_CORES
    mesh = Mesh(np.asarray(devices), ("core",))
    donate = tuple(range(n_params, n_params + len(out_names)))
    sharded = jax.jit(
        shard_map(
            _body, mesh=mesh,
            in_specs=(PartitionSpec("core"),) * (n_params + len(out_names)),
            out_specs=(PartitionSpec("core"),) * len(out_names),
            check_rep=False),
        donate_argnums=donate, keep_unused=True)

    def run(in_maps):
        per_core = [[np.asarray(m[name]) for name in in_names]
                    for m in in_maps]
        concat_in = [
            np.concatenate([per_core[c][i] for c in range(N_CORES)], axis=0)
            for i in range(n_params)
        ]
        zero_outs = [np.zeros((N_CORES * s[0],) + s[1:], d)
                     for s, d in zip(out_shapes, out_dtypes)]
        outs = sharded(*concat_in, *zero_outs)
        results = []
        for c in range(N_CORES):
            m = {}
            for i, name in enumerate(out_names):
                s0 = out_shapes[i][0]
                m[name] = np.asarray(outs[i][c * s0:(c + 1) * s0])
            results.append(m)
        return results

    _RUNNER = run
    return run


def kernel(x, weight_mu, weight_log_var, bias_mu, bias_log_var,
           weight_eps, bias_eps):
    in_maps = make_in_maps(x, weight_mu, weight_log_var, bias_mu,
                           bias_log_var, weight_eps, bias_eps)
    results = _get_runner()(in_maps)
    return gather_output(results)


# revision 17
# speedup vs baseline: 1.0952x; 1.0007x over previous
"""BayesianLinear (reparameterized sampling + linear) on 8 TRN2 NeuronCores.

Math:  w = weight_mu + weight_eps * exp(0.5*weight_log_var)   [OUT_F, IN_F]
       b = bias_mu + bias_eps * exp(0.5*bias_log_var)         [OUT_F]
       out = x @ w.T + b                                      [BATCH, OUT_F]

Sharding: 2-way over BATCH x 4-way over OUT_F (8 cores).  Each core gets
K-major (transposed) bf16 copies of its x / weight shards, samples the
weight on device into an SBUF-resident bf16 [IN_F, O_CORE] tensor, and
streams x through the TensorEngine in bf16 (fp32 PSUM accumulate).
Bias is sampled on device in fp32 and fused into the PSUM->SBUF copy.
"""

import numpy as np
import ml_dtypes

BATCH = 8192
IN_F = 4096
OUT_F = 4096
B_SHARDS = 2
O_SHARDS = 4
N_CORES = B_SHARDS * O_SHARDS

B_CORE = BATCH // B_SHARDS   # 4096
O_CORE = OUT_F // O_SHARDS   # 1024

BF16 = ml_dtypes.bfloat16

_PROGRAM_CACHE = {}


def build_program(B_core=B_CORE, O_core=O_CORE, K=IN_F):
    """Build + compile the per-core Bass/Tile program (same NEFF on all cores).

    DRAM parameters (per core):
      xt   [K, B_core]  bf16   x shard, transposed (K-major)
      wmu  [K, O_core]  bf16   weight_mu shard, transposed
      wlv  [K, O_core]  bf16   weight_log_var shard, transposed
      weps [K, O_core]  bf16   weight_eps shard, transposed
      bmu/blv/beps [1, O_core] f32
      out  [B_core, O_core] f32
    """
    import concourse.mybir as mybir
    import concourse.tile as tile
    from concourse import bacc

    assert K % 128 == 0 and B_core % 512 == 0 and O_core % 512 == 0
    KT = K // 128          # contraction tiles
    MT = B_core // 512     # x blocks (512 batch rows each)
    NO = O_core // 512     # output free-dim chunks

    f32 = mybir.dt.float32
    bf16 = mybir.dt.bfloat16
    Exp = mybir.ActivationFunctionType.Exp
    mult = mybir.AluOpType.mult
    add = mybir.AluOpType.add

    nc = bacc.Bacc("TRN2", target_bir_lowering=False, debug=False)

    xt = nc.declare_dram_parameter("xt", [K, B_core], bf16, isOutput=False)
    wmu = nc.declare_dram_parameter("wmu", [K, O_core], bf16, isOutput=False)
    wlv = nc.declare_dram_parameter("wlv", [K, O_core], bf16, isOutput=False)
    weps = nc.declare_dram_parameter("weps", [K, O_core], bf16, isOutput=False)
    bmu = nc.declare_dram_parameter("bmu", [1, O_core], f32, isOutput=False)
    blv = nc.declare_dram_parameter("blv", [1, O_core], f32, isOutput=False)
    beps = nc.declare_dram_parameter("beps", [1, O_core], f32, isOutput=False)
    out = nc.declare_dram_parameter("out", [B_core, O_core], f32, isOutput=True)

    xt_r = xt.ap().rearrange("(kt p) b -> p kt b", p=128)
    out_r = out.ap().rearrange("(mt p) o -> p mt o", p=128)

    KC = 4                  # k-tiles per x chunk (batched DMA)
    NC_CHUNKS = KT // KC
    X_BUFS = 2
    # Weight-stage chunk sizes (in k-tiles). Small leading chunks get the
    # first matmuls started ~15 us earlier; 4-tile chunks amortize the
    # ~1 us/dma_start enqueue cost for the bulk of the stream.
    WSIZES = [1, 1, 2] + [4] * ((KT - 4) // 4) if KT >= 8 else [1] * KT
    assert sum(WSIZES) == KT
    WSTARTS = [sum(WSIZES[:i]) for i in range(len(WSIZES))]
    K2C = []
    for ci, (s, st) in enumerate(zip(WSIZES, WSTARTS)):
        K2C += [(ci, k - st) for k in range(st, st + s)]

    with tile.TileContext(nc) as tc:
        with (
            tc.tile_pool(name="wres", bufs=1) as wres_pool,
            tc.tile_pool(name="wstage", bufs=2) as wstage_pool,
            tc.tile_pool(name="xblk", bufs=X_BUFS * NC_CHUNKS) as xpool,
            tc.tile_pool(name="osb", bufs=3) as opool,
            tc.tile_pool(name="bias", bufs=1) as bias_pool,
            tc.tile_pool(name="psum", bufs=8, space="PSUM") as ppool,
        ):
            # ---- bias: b = bmu + beps * exp(0.5*blv), replicated to 128
            # partitions. Emitted lazily (after the first weight chunks) so
            # its DMAs stay off the startup critical path — it is only
            # needed by the first copy_out, ~80 us in.
            def emit_bias():
                bstage = bias_pool.tile([1, 3 * O_core], f32, tag="bstage",
                                        name="bstage")
                b_lv_t = bstage[:, 0:O_core]
                b_eps_t = bstage[:, O_core:2 * O_core]
                b_mu_t = bstage[:, 2 * O_core:3 * O_core]
                nc.sync.dma_start(out=b_lv_t, in_=blv.ap())
                nc.sync.dma_start(out=b_eps_t, in_=beps.ap())
                nc.sync.dma_start(out=b_mu_t, in_=bmu.ap())
                nc.scalar.activation(b_lv_t, b_lv_t, Exp, scale=0.5)
                nc.vector.tensor_tensor(out=b_lv_t, in0=b_lv_t,
                                        in1=b_eps_t, op=mult)
                nc.vector.tensor_tensor(out=b_lv_t, in0=b_lv_t,
                                        in1=b_mu_t, op=add)
                rep = bias_pool.tile([128, O_core], f32, tag="brep",
                                     name="brep")
                nc.gpsimd.partition_broadcast(rep[:], b_lv_t)
                return rep

            # x is loaded per (block, k-chunk) so the first matmuls only wait
            # for one 0.5 MB x chunk + one weight chunk, not a whole block.
            wmu_r = wmu.ap().rearrange("(c p) o -> p c o", p=128)
            wlv_r = wlv.ap().rearrange("(c p) o -> p c o", p=128)
            weps_r = weps.ap().rearrange("(c p) o -> p c o", p=128)
            xt_c = xt.ap().rearrange("(c kc p) b -> p c kc b", p=128, kc=KC)

            def load_x_chunk(m, c):
                t = xpool.tile([128, KC, 512], bf16, tag="xblk",
                               name=f"xblk_{m}_{c}")
                nc.sync.dma_start(out=t[:],
                                  in_=xt_c[:, c, :, m * 512:(m + 1) * 512])
                return t

            def load_w_chunk(ci):
                size, st = WSIZES[ci], WSTARTS[ci]
                ksl = slice(st, st + size)
                # DMA order = critical-path order: exp needs lv first,
                # then mult needs eps, add needs mu last.
                slv = wstage_pool.tile([128, size, O_core], bf16, tag="slv",
                                       name=f"slv_{ci}")
                seps = wstage_pool.tile([128, size, O_core], bf16, tag="seps",
                                        name=f"seps_{ci}")
                smu = wstage_pool.tile([128, size, O_core], bf16, tag="smu",
                                       name=f"smu_{ci}")
                nc.sync.dma_start(out=slv[:], in_=wlv_r[:, ksl, :])
                nc.sync.dma_start(out=seps[:], in_=weps_r[:, ksl, :])
                nc.sync.dma_start(out=smu[:], in_=wmu_r[:, ksl, :])
                small = size < max(WSIZES)
                w_c = wres_pool.tile(
                    [128, size, O_core], bf16,
                    tag="wres_s" if small else "wres",
                    bufs=(sum(1 for s in WSIZES if s < max(WSIZES)) if small
                          else sum(1 for s in WSIZES if s == max(WSIZES))),
                    name=f"wres_{ci}")
                nc.scalar.activation(w_c[:], slv[:], Exp, scale=0.5)
                nc.vector.tensor_tensor(out=w_c[:], in0=w_c[:], in1=seps[:], op=mult)
                nc.vector.tensor_tensor(out=w_c[:], in0=w_c[:], in1=smu[:], op=add)
                return w_c

            # Interleave m0's x chunks with the weight chunks in the DMA
            # queue, aligned so MMs at k have both x0[k] and w[k] as early
            # as possible.
            xtiles = {0: []}
            wchunks = []
            wchunks.append(load_w_chunk(0))   # w0 first: longest dep chain
            next_w = 1
            bias_rep = None
            for c in range(NC_CHUNKS):
                xtiles[0].append(load_x_chunk(0, c))
                want_k = (c + 1) * KC  # w coverage needed for x chunks 0..c
                while next_w < len(WSIZES) and WSTARTS[next_w] < want_k:
                    wchunks.append(load_w_chunk(next_w))
                    next_w += 1
                if bias_rep is None:
                    bias_rep = emit_bias()
            while next_w < len(WSIZES):
                wchunks.append(load_w_chunk(next_w))
                next_w += 1
            if MT > 1:
                xtiles[1] = [load_x_chunk(1, c) for c in range(NC_CHUNKS)]

            def wres_slice(k, n):
                ci, off = K2C[k]
                return wchunks[ci][:, off, n * 512:(n + 1) * 512]

            # ---- matmul: out[b, o] = x @ w.T + bias
            def emit_group(m, psum, xchunks, ms, n, k_range):
                for k in k_range:
                    nc.tensor.matmul(
                        psum[:],
                        xchunks[k // KC][:, k % KC, ms * 128:(ms + 1) * 128],
                        wres_slice(k, n),
                        start=(k == 0),
                        stop=(k == KT - 1),
                    )

            def copy_out(m, psum, ms, n):
                osb = opool.tile([128, 512], f32, tag="osb",
                                 name=f"osb_{m}_{ms}_{n}")
                nc.vector.tensor_tensor(
                    out=osb[:], in0=psum[:],
                    in1=bias_rep[:, n * 512:(n + 1) * 512], op=add)
                nc.sync.dma_start(
                    out=out_r[:, m * 4 + ms, n * 512:(n + 1) * 512],
                    in_=osb[:])

            for m in range(MT):
                if m in xtiles:
                    xchunks = xtiles.pop(m)
                else:
                    xchunks = [load_x_chunk(m, c) for c in range(NC_CHUNKS)]
                if m == 0:
                    # k-outer: all 8 psum groups accumulate in lockstep with
                    # the arriving weight stream.
                    psums = [
                        [ppool.tile([128, 512], f32, tag="ps",
                                    name=f"ps_{m}_{ms}_{n}") for n in range(NO)]
                        for ms in range(4)
                    ]
                    for k in range(KT):
                        for ms in range(4):
                            for n in range(NO):
                                emit_group(m, psums[ms][n], xchunks, ms, n, [k])
                    for ms in range(4):
                        for n in range(NO):
                            copy_out(m, psums[ms][n], ms, n)
                else:
                    # k-inner: each group finishes + copies out immediately,
                    # overlapping the next group's matmuls.
                    for ms in range(4):
                        for n in range(NO):
                            psum = ppool.tile([128, 512], f32, tag="ps",
                                              name=f"ps_{m}_{ms}_{n}")
                            emit_group(m, psum, xchunks, ms, n, range(KT))
                            copy_out(m, psum, ms, n)

    nc.compile()
    return nc


def _get_program():
    key = (B_CORE, O_CORE, IN_F)
    if key not in _PROGRAM_CACHE:
        _PROGRAM_CACHE[key] = build_program(*key)
    return _PROGRAM_CACHE[key]


def make_in_maps(x, weight_mu, weight_log_var, bias_mu, bias_log_var,
                 weight_eps, bias_eps):
    """Shard + transpose + cast the full inputs into 8 per-core input maps."""
    x = np.asarray(x, dtype=np.float32)
    weight_mu = np.asarray(weight_mu, dtype=np.float32)
    weight_log_var = np.asarray(weight_log_var, dtype=np.float32)
    weight_eps = np.asarray(weight_eps, dtype=np.float32)
    bias_mu = np.asarray(bias_mu, dtype=np.float32).reshape(-1)
    bias_log_var = np.asarray(bias_log_var, dtype=np.float32).reshape(-1)
    bias_eps = np.asarray(bias_eps, dtype=np.float32).reshape(-1)

    xt = np.ascontiguousarray(x.astype(BF16).T)              # [IN_F, BATCH]
    wmu_t = np.ascontiguousarray(weight_mu.astype(BF16).T)   # [IN_F, OUT_F]
    wlv_t = np.ascontiguousarray(weight_log_var.astype(BF16).T)
    weps_t = np.ascontiguousarray(weight_eps.astype(BF16).T)

    in_maps = []
    for c in range(N_CORES):
        bi, oi = divmod(c, O_SHARDS)
        bs = slice(bi * B_CORE, (bi + 1) * B_CORE)
        os_ = slice(oi * O_CORE, (oi + 1) * O_CORE)
        in_maps.append({
            "xt": np.ascontiguousarray(xt[:, bs]),
            "wmu": np.ascontiguousarray(wmu_t[:, os_]),
            "wlv": np.ascontiguousarray(wlv_t[:, os_]),
            "weps": np.ascontiguousarray(weps_t[:, os_]),
            "bmu": np.ascontiguousarray(bias_mu[os_]).reshape(1, O_CORE),
            "blv": np.ascontiguousarray(bias_log_var[os_]).reshape(1, O_CORE),
            "beps": np.ascontiguousarray(bias_eps[os_]).reshape(1, O_CORE),
        })
    return in_maps


def gather_output(results):
    out = np.empty((BATCH, OUT_F), dtype=np.float32)
    for c in range(N_CORES):
        bi, oi = divmod(c, O_SHARDS)
        out[bi * B_CORE:(bi + 1) * B_CORE, oi * O_CORE:(oi + 1) * O_CORE] = \
            results[c]["out"]
    return out


def run_on_hw(in_maps, trace=False):
    from concourse.bass_utils import run_bass_kernel_spmd
    nc = _get_program()
    return run_bass_kernel_spmd(nc, in_maps, list(range(N_CORES)), trace=trace)


_RUNNER = None


def _get_runner():
    """Build (once per process) a cached jit callable: in_maps -> results.

    Mirrors bass2jax.run_bass_via_pjrt's multi-core branch, but keeps the
    jitted executable alive so repeated kernel() calls skip recompilation.
    """
    global _RUNNER
    if _RUNNER is not None:
        return _RUNNER
    import jax
    from jax.sharding import Mesh, PartitionSpec
    try:
        from jax.experimental.shard_map import shard_map
    except ImportError:  # newer jax
        from jax import shard_map
    import concourse.mybir as mybir
    from concourse import bass2jax

    nc = _get_program()
    bass2jax.install_neuronx_cc_hook()
    assert nc.dbg_addr is None and nc.partition_id_tensor is None

    in_names, out_names, out_shapes, out_dtypes = [], [], [], []
    for alloc in nc.m.functions[0].allocations:
        if not isinstance(alloc, mybir.MemoryLocationSet):
            continue
        name = alloc.memorylocations[0].name
        if alloc.kind == "ExternalInput":
            in_names.append(name)
        elif alloc.kind == "ExternalOutput":
            out_names.append(name)
            out_shapes.append(tuple(alloc.tensor_shape))
            out_dtypes.append(mybir.dt.np(alloc.dtype))
    out_avals = [jax.core.ShapedArray(s, d)
                 for s, d in zip(out_shapes, out_dtypes)]
    n_params = len(in_names)
    all_names = tuple(in_names + out_names)

    def _body(*args):
        outs = bass2jax._bass_exec_p.bind(
            *args,
            out_avals=tuple(out_avals),
            in_names=all_names,
            out_names=tuple(out_names),
            lowering_input_output_aliases=(),
            sim_require_finite=True,
            sim_require_nnan=True,
            nc=nc,
        )
        return tuple(outs)

    devices = jax.devices()[:N_CORES]
    assert len(devices) == N_CORES
    mesh = Mesh(np.asarray(devices), ("core",))
    donate = tuple(range(n_params, n_params + len(out_names)))
    sharded = jax.jit(
        shard_map(
            _body, mesh=mesh,
            in_specs=(PartitionSpec("core"),) * (n_params + len(out_names)),
            out_specs=(PartitionSpec("core"),) * len(out_names),
            check_rep=False),
        donate_argnums=donate, keep_unused=True)

    def run(in_maps):
        per_core = [[np.asarray(m[name]) for name in in_names]
                    for m in in_maps]
        concat_in = [
            np.concatenate([per_core[c][i] for c in range(N_CORES)], axis=0)
            for i in range(n_params)
        ]
        zero_outs = [np.zeros((N_CORES * s[0],) + s[1:], d)
                     for s, d in zip(out_shapes, out_dtypes)]
        outs = sharded(*concat_in, *zero_outs)
        results = []
        for c in range(N_CORES):
            m = {}
            for i, name in enumerate(out_names):
                s0 = out_shapes[i][0]
                m[name] = np.asarray(outs[i][c * s0:(c + 1) * s0])
            results.append(m)
        return results

    _RUNNER = run
    return run


def kernel(x, weight_mu, weight_log_var, bias_mu, bias_log_var,
           weight_eps, bias_eps):
    in_maps = make_in_maps(x, weight_mu, weight_log_var, bias_mu,
                           bias_log_var, weight_eps, bias_eps)
    results = _get_runner()(in_maps)
    return gather_output(results)
